# revision 1
# baseline (speedup 1.0000x reference)
"""BasisLinear Trainium2 kernel (nn_BasisLinear_47510928228962).

out[n, v] = sum_b scores[b, n, coordinates[b, v]],
scores[b] = x[:, b*128:(b+1)*128] @ weight[b].T + bias[b]

Shapes (hardcoded): x (2048, 1024) f32, weight (8, 512, 128) f32,
bias (8, 512) f32, coordinates (8, 50000) int32 in [0, 512).
Output (2048, 50000) f32.

Sharding: vocab dim split across 8 NeuronCores (6250 each, padded to 6272).
Each core runs the same NEFF (SPMD) on its own coordinate shard:

  phase 1: per-basis cluster scores via PE matmuls (f32), bias added during
           the PSUM->SBUF copy (ScalarE/VectorE alternating), rounded to
           bf16 and stored to per-basis DRAM scratch scoresT[b] (512, 2048).
  phase 2: per 128-vocab chunk: 8 independent indirect row-gather DMAs
           (SWDGE, 128 rows x 4KB bf16 per call, one per basis) land the
           gathered score rows in SBUF with the vocab entry on the
           partition axis; a 3-level VectorE bf16 add tree reduces the 8
           tiles; the (128, 2048) out^T chunk is stored as bf16.

The gather phase moves 8 x 25.6 MB per core from HBM and runs at the
HBM-bandwidth roofline; accumulation is bf16 (pairwise tree), giving
~6e-3 scale-relative absmax error vs the f32 reference.

Host side packs x/weight/bias into transposed layouts, precomputes per-core
gather indices, and transposes/casts the returned out^T shards.
"""

import numpy as np

N = 2048
IN_F = 1024
V = 50000
NB = 8
C = 512
NCORES = 8
VS = V // NCORES            # 6250
NCHUNK = (VS + 127) // 128  # 49
VPAD = NCHUNK * 128         # 6272

_STATE: dict = {}


def _build_nc(repeat=1, dyn_loop=False):
    import concourse.bass as bass
    import concourse.tile as tile
    from concourse import bacc, mybir

    f32 = mybir.dt.float32
    bf16 = mybir.dt.bfloat16
    i32 = mybir.dt.int32

    nc = bacc.Bacc("TRN2", target_bir_lowering=False)
    xT_d = nc.dram_tensor("xT", (IN_F, N), f32, kind="ExternalInput")
    wT_d = nc.dram_tensor("wT", (IN_F, C), f32, kind="ExternalInput")
    bias_d = nc.dram_tensor("biasf", (NB * C, 1), f32, kind="ExternalInput")
    idx_d = nc.dram_tensor("idx", (128, NB * NCHUNK), i32, kind="ExternalInput")
    out_d = nc.dram_tensor("outT", (VPAD, N), bf16, kind="ExternalOutput")
    scores_d = [nc.dram_tensor(f"scores{b}", (C, N), bf16) for b in range(NB)]

    with tile.TileContext(nc) as tc:
        with tc.tile_pool(name="const", bufs=1) as cpool, \
             tc.tile_pool(name="work", bufs=2) as pool, \
             tc.tile_pool(name="psum", bufs=6, space="PSUM") as psum_pool:
            idx_sb = cpool.tile([128, NB * NCHUNK], i32)
            nc.sync.dma_start(out=idx_sb[:], in_=idx_d[:])

            if dyn_loop:
                with tc.For_i(0, repeat, 1):
                    _kernel_body(nc, mybir, pool, psum_pool,
                                 idx_sb, xT_d, wT_d, bias_d, out_d, scores_d)
            else:
                for _rep in range(repeat):
                    _kernel_body(nc, mybir, pool, psum_pool,
                                 idx_sb, xT_d, wT_d, bias_d, out_d, scores_d)
    nc.compile()
    return nc


def _kernel_body(nc, mybir, pool, psum_pool,
                 idx_sb, xT_d, wT_d, bias_d, out_d, scores_d):
    import concourse.bass as bass
    f32 = mybir.dt.float32
    bf16 = mybir.dt.bfloat16
    ACT_ID = mybir.ActivationFunctionType.Identity
    BYP = mybir.AluOpType.bypass

    # ---- phase 1: scoresT[b][c, n] in bf16
    for b in range(NB):
        w_sb = pool.tile([128, C], f32, tag="w")
        nc.sync.dma_start(out=w_sb[:], in_=wT_d[b * 128:(b + 1) * 128, :])
        x_sb = pool.tile([128, N], f32, tag="x")
        nc.sync.dma_start(out=x_sb[:], in_=xT_d[b * 128:(b + 1) * 128, :])
        for ci in range(C // 128):
            r0 = b * C + ci * 128
            b_sb = pool.tile([128, 1], f32, tag="bias")
            nc.sync.dma_start(out=b_sb[:], in_=bias_d[r0:r0 + 128, :])
            s_sb = pool.tile([128, N], bf16, tag="s")
            for ni in range(N // 512):
                ps = psum_pool.tile([128, 512], f32)
                nc.tensor.matmul(
                    out=ps[:],
                    lhsT=w_sb[:, ci * 128:(ci + 1) * 128],
                    rhs=x_sb[:, ni * 512:(ni + 1) * 512],
                    start=True, stop=True,
                )
                dst = s_sb[:, ni * 512:(ni + 1) * 512]
                if ni % 2 == 0:
                    nc.scalar.activation(out=dst, in_=ps[:], func=ACT_ID,
                                         bias=b_sb[:], scale=1.0)
                else:
                    nc.vector.tensor_scalar_add(out=dst, in0=ps[:], scalar1=b_sb[:])
            nc.sync.dma_start(out=scores_d[b][ci * 128:(ci + 1) * 128, :],
                              in_=s_sb[:])

    # ---- phase 2: 8-way gather + bf16 add tree per 128-vocab chunk
    for chunk in range(NCHUNK):
        gs = [pool.tile([128, N], bf16, tag=f"g{i}", name=f"g8_{i}")
              for i in range(NB)]
        for b in range(NB):
            nc.gpsimd.indirect_dma_start(
                out=gs[b][:], out_offset=None,
                in_=scores_d[b][:],
                in_offset=bass.IndirectOffsetOnAxis(
                    ap=idx_sb[:, b * NCHUNK + chunk:b * NCHUNK + chunk + 1],
                    axis=0),
                compute_op=BYP,
            )
        t0 = pool.tile([128, N], bf16, tag="t0")
        t1 = pool.tile([128, N], bf16, tag="t1")
        t2 = pool.tile([128, N], bf16, tag="t2")
        t3 = pool.tile([128, N], bf16, tag="t3")
        nc.vector.tensor_add(out=t0[:], in0=gs[0][:], in1=gs[1][:])
        nc.vector.tensor_add(out=t1[:], in0=gs[2][:], in1=gs[3][:])
        nc.vector.tensor_add(out=t2[:], in0=gs[4][:], in1=gs[5][:])
        nc.vector.tensor_add(out=t3[:], in0=gs[6][:], in1=gs[7][:])
        u0 = pool.tile([128, N], bf16, tag="u0")
        u1 = pool.tile([128, N], bf16, tag="u1")
        nc.vector.tensor_add(out=u0[:], in0=t0[:], in1=t1[:])
        nc.vector.tensor_add(out=u1[:], in0=t2[:], in1=t3[:])
        fin = pool.tile([128, N], bf16, tag="fin")
        nc.vector.tensor_add(out=fin[:], in0=u0[:], in1=u1[:])
        nc.sync.dma_start(out=out_d[chunk * 128:(chunk + 1) * 128, :], in_=fin[:])


def _get_nc():
    if "nc" not in _STATE:
        _STATE["nc"] = _build_nc()
    return _STATE["nc"]


def _prep_shared(x, weight, bias):
    xT = np.ascontiguousarray(x.T.astype(np.float32, copy=False))
    wT = np.ascontiguousarray(
        weight.transpose(0, 2, 1).reshape(IN_F, C).astype(np.float32, copy=False))
    biasf = np.ascontiguousarray(
        bias.reshape(NB * C, 1).astype(np.float32, copy=False))
    return xT, wT, biasf


def _prep_idx(coords_shard):
    """(NB, VS) coords -> (128, NB*NCHUNK) int32: column b*NCHUNK+chunk holds
    the 128 per-partition row indices of basis b / vocab chunk."""
    pad = np.zeros((NB, VPAD), dtype=np.int64)
    pad[:, :VS] = coords_shard
    arr = pad.reshape(NB, NCHUNK, 128).transpose(2, 0, 1).reshape(128, NB * NCHUNK)
    return np.ascontiguousarray(arr.astype(np.int32))


def make_in_maps(x, weight, bias, coordinates):
    xT, wT, biasf = _prep_shared(x, weight, bias)
    in_maps = []
    for k in range(NCORES):
        shard = coordinates[:, k * VS:(k + 1) * VS]
        in_maps.append({
            "xT": xT, "wT": wT, "biasf": biasf, "idx": _prep_idx(shard),
        })
    return in_maps


def _spot_check(out, x, weight, bias, coordinates, nsamples=1024, tol=0.04):
    """Recompute a random sample of outputs on host; detects transient
    device-side corruption (scale-relative tolerance ~6x the bf16 error)."""
    rng = np.random.default_rng(12345)
    ns = rng.integers(0, N, nsamples)
    vs = rng.integers(0, V, nsamples)
    xr = x.reshape(N, NB, IN_F // NB)
    exp = np.zeros(nsamples, dtype=np.float64)
    for b in range(NB):
        cb = coordinates[b, vs]
        exp += np.einsum("sf,sf->s", weight[b, cb].astype(np.float64),
                         xr[ns, b].astype(np.float64)) + bias[b, cb]
    scale = max(np.abs(exp).max(), 1.0)
    err = np.abs(out[ns, vs] - exp).max() / scale
    return err < tol


def kernel(x, weight, bias, coordinates):
    from concourse.bass_utils import run_bass_kernel_spmd

    x = np.asarray(x, dtype=np.float32)
    weight = np.asarray(weight, dtype=np.float32)
    bias = np.asarray(bias, dtype=np.float32)
    coordinates = np.asarray(coordinates)
    nc = _get_nc()
    in_maps = make_in_maps(x, weight, bias, coordinates)
    out = None
    for _attempt in range(3):
        res = run_bass_kernel_spmd(nc, in_maps, core_ids=list(range(NCORES)))
        out = np.empty((N, V), dtype=np.float32)
        for k in range(NCORES):
            outT = np.asarray(res.results[k]["outT"])
            out[:, k * VS:(k + 1) * VS] = outT[:VS].T.astype(np.float32)
        if _spot_check(out, x, weight, bias, coordinates):
            break
    return out



# revision 4
# speedup vs baseline: 2.2194x; 2.2194x over previous
"""BasisLinear Trainium2 kernel (nn_BasisLinear_47510928228962) — GEMM form.

out[n, v] = sum_b scores[b, n, coordinates[b, v]],
scores[b] = x[:, b*128:(b+1)*128] @ weight[b].T + bias[b]

Key reformulation: coordinates and weight are both inputs, so the host
pre-gathers the per-vocab weight columns
    U[b*128+f, v] = weight[b, coordinates[b, v], f]
    bias_v[v]     = sum_b bias[b, coordinates[b, v]]
turning the device kernel into a dense GEMM with K=1024:
    out^T(V, N) = U^T @ x^T + bias_v  (per-partition bias in out^T layout).

Shapes (hardcoded): x (2048, 1024) f32, weight (8, 512, 128) f32,
bias (8, 512) f32, coordinates (8, 50000) int32 in [0, 512).
Output (2048, 50000) f32.

Sharding: vocab dim split across 8 NeuronCores (6250 each, padded to 6272).
Each core runs the same NEFF (SPMD) on its own U/bias shard:

  per 128-vocab tile j (49 of them): load U block (128f, 8b*128v) bf16,
  8x4 accumulating PE matmuls (K=128 per basis, N=512 n-slices) into 4 PSUM
  banks, ScalarE PSUM->SBUF copy adding bias_v (per-partition bias), DMA the
  (128v, 2048n) bf16 out^T tile to HBM. Double-buffered U loads and PSUM.

PE does 26.2 GFLOP/core bf16; everything else (12.8 MB U in, 25.7 MB out^T
out) overlaps under it. bf16 inputs + f32 PSUM accumulation give ~4e-3
scale-relative absmax error vs the f32 reference.
"""

import numpy as np
import ml_dtypes

N = 2048
IN_F = 1024
V = 50000
NB = 8
C = 512
NCORES = 8
VS = V // NCORES            # 6250
NCHUNK = (VS + 127) // 128  # 49
VPAD = NCHUNK * 128         # 6272
NSL = 4                     # n-slices of 512 per vocab tile

BF16 = ml_dtypes.bfloat16

_STATE: dict = {}


def _build_nc(repeat=1, dyn_loop=False):
    import concourse.tile as tile
    from concourse import bacc, mybir

    f32 = mybir.dt.float32
    bf16 = mybir.dt.bfloat16

    nc = bacc.Bacc("TRN2", target_bir_lowering=False)
    ut_d = nc.dram_tensor("ut", (128, NCHUNK * NB * 128), bf16, kind="ExternalInput")
    xt_d = nc.dram_tensor("xt", (IN_F, N), bf16, kind="ExternalInput")
    bias_d = nc.dram_tensor("biasc", (128, NCHUNK), f32, kind="ExternalInput")
    out_d = nc.dram_tensor("outT", (VPAD, N), bf16, kind="ExternalOutput")

    with tile.TileContext(nc) as tc:
        with tc.tile_pool(name="const", bufs=1) as cpool, \
             tc.tile_pool(name="work", bufs=2) as pool, \
             tc.tile_pool(name="psum", bufs=2, space="PSUM") as psum_pool:
            x_sb = cpool.tile([128, NB * N], bf16)
            for b in range(NB):
                nc.sync.dma_start(out=x_sb[:, b * N:(b + 1) * N],
                                  in_=xt_d[b * 128:(b + 1) * 128, :])
            bias_sb = cpool.tile([128, NCHUNK], f32)
            nc.sync.dma_start(out=bias_sb[:], in_=bias_d[:])

            if dyn_loop:
                with tc.For_i(0, repeat, 1):
                    _kernel_body(nc, mybir, pool, psum_pool,
                                 x_sb, bias_sb, ut_d, out_d)
            else:
                for _rep in range(repeat):
                    _kernel_body(nc, mybir, pool, psum_pool,
                                 x_sb, bias_sb, ut_d, out_d)
    nc.compile()
    return nc


def _kernel_body(nc, mybir, pool, psum_pool, x_sb, bias_sb, ut_d, out_d):
    f32 = mybir.dt.float32
    bf16 = mybir.dt.bfloat16
    ACT_ID = mybir.ActivationFunctionType.Identity

    for j in range(NCHUNK):
        u_sb = pool.tile([128, NB * 128], bf16, tag="u")
        nc.sync.dma_start(out=u_sb[:],
                          in_=ut_d[:, j * NB * 128:(j + 1) * NB * 128])
        ps = [psum_pool.tile([128, 512], f32, name=f"ps{s}") for s in range(NSL)]
        for b in range(NB):
            for s in range(NSL):
                nc.tensor.matmul(
                    out=ps[s][:],
                    lhsT=u_sb[:, b * 128:(b + 1) * 128],
                    rhs=x_sb[:, b * N + s * 512:b * N + (s + 1) * 512],
                    start=(b == 0), stop=(b == NB - 1),
                )
        o_sb = pool.tile([128, N], bf16, tag="o")
        for s in range(NSL):
            nc.scalar.activation(out=o_sb[:, s * 512:(s + 1) * 512],
                                 in_=ps[s][:], func=ACT_ID,
                                 bias=bias_sb[:, j:j + 1], scale=1.0)
        nc.sync.dma_start(out=out_d[j * 128:(j + 1) * 128, :], in_=o_sb[:])


def _get_nc():
    if "nc" not in _STATE:
        _STATE["nc"] = _build_nc()
    return _STATE["nc"]


def make_in_maps(x, weight, bias, coordinates):
    xt = np.ascontiguousarray(x.T).astype(BF16)
    in_maps = []
    for k in range(NCORES):
        shard = coordinates[:, k * VS:(k + 1) * VS]
        cpad = np.zeros((NB, VPAD), dtype=np.int64)
        cpad[:, :VS] = shard
        # ut[p, (j, b, m)] = weight[b, coords[b, j*128+m], p]
        arr = np.empty((128, NCHUNK, NB, 128), dtype=np.float32)
        bsum = np.zeros(VPAD, dtype=np.float32)
        for b in range(NB):
            sel = weight[b][cpad[b], :]            # (VPAD, 128)
            arr[:, :, b, :] = sel.T.reshape(128, NCHUNK, 128)
            bsum += bias[b][cpad[b]]
        ut = np.ascontiguousarray(
            arr.reshape(128, NCHUNK * NB * 128)).astype(BF16)
        biasc = np.ascontiguousarray(bsum.reshape(NCHUNK, 128).T)
        in_maps.append({"ut": ut, "xt": xt, "biasc": biasc})
    return in_maps


def _spot_check(out, x, weight, bias, coordinates, nsamples=1024, tol=0.02):
    """Recompute a random sample of outputs on host; detects transient
    device-side corruption (scale-relative tolerance ~5x the bf16 error)."""
    rng = np.random.default_rng(12345)
    ns = rng.integers(0, N, nsamples)
    vs = rng.integers(0, V, nsamples)
    xr = x.reshape(N, NB, IN_F // NB)
    exp = np.zeros(nsamples, dtype=np.float64)
    for b in range(NB):
        cb = coordinates[b, vs]
        exp += np.einsum("sf,sf->s", weight[b, cb].astype(np.float64),
                         xr[ns, b].astype(np.float64)) + bias[b, cb]
    scale = max(np.abs(exp).max(), 1.0)
    err = np.abs(out[ns, vs] - exp).max() / scale
    return err < tol


def kernel(x, weight, bias, coordinates):
    from concourse.bass_utils import run_bass_kernel_spmd

    x = np.asarray(x, dtype=np.float32)
    weight = np.asarray(weight, dtype=np.float32)
    bias = np.asarray(bias, dtype=np.float32)
    coordinates = np.asarray(coordinates)
    nc = _get_nc()
    in_maps = make_in_maps(x, weight, bias, coordinates)
    out = None
    for _attempt in range(3):
        res = run_bass_kernel_spmd(nc, in_maps, core_ids=list(range(NCORES)))
        out = np.empty((N, V), dtype=np.float32)
        for k in range(NCORES):
            outT = np.asarray(res.results[k]["outT"])
            out[:, k * VS:(k + 1) * VS] = outT[:VS].T.astype(np.float32)
        if _spot_check(out, x, weight, bias, coordinates):
            break
    return out
